# revision 6
# baseline (speedup 1.0000x reference)
"""ConvCNP encoder kernel for 8 Trainium2 NeuronCores.

Computes, for full inputs X(4,1024,2), Y(4,1024,2), grid(16384,2):
    Gram = exp(-0.5*||grid-X||^2)          (B, G, n)
    FM   = Gram @ [1, Y]                   (B, G, 3)
    out  = [FM0, FM1/FM0, FM2/FM0] -> (B, 3, 128, 128)  (y, x image axes)

Sharding: grid axis G split 8 ways (2048 rows / core = 16 output
x-columns); every core handles all 4 batches; no cross-device reduction.

Per-core device pipeline:
  mm1 (PE):  -0.5*d2 as a K=10 bf16 matmul using hi/lo split of
             g.x - 0.5|g|^2 - 0.5|x|^2  -> PSUM [n-tile 128, g 512]
  exp (ACT): PSUM -> SBUF Gram (bf16), batched [128, 3*512] activations
  mm2 (PE):  Gram[n,g128] stationary x E[n, (1,Yhi,Ylo)] moving,
             accumulated over 8 n-tiles -> FM [g(=y) 128, 5] PSUM
  norm (DVE): density reciprocal + multiplies, fp32
  DMA out:   [y, x] tiles per (b, c)
"""

import numpy as np

B = 4
N = 1024
G = 16384
NCORES = 8
GS = G // NCORES          # 2048 grid rows per core
NT = N // 128             # 8 context tiles
JS = GS // 512            # 4 g-blocks of 512 per core
SLOTS = NT * JS           # 32 mm1 slots (each [128, 512]) per batch
K = 10                    # contraction rows of the d2 factorization
XCOLS = GS // 128         # 16 output x-columns per core

_CACHE = {}


def _build_nc():
    import concourse.bacc as bacc
    import concourse.mybir as mybir
    import concourse.tile as tile
    from contextlib import ExitStack

    f32 = mybir.dt.float32
    bf16 = mybir.dt.bfloat16

    nc = bacc.Bacc("TRN2", target_bir_lowering=False, debug=False,
                   num_devices=NCORES)
    ax_d = nc.dram_tensor("Ax", [K, B, NT, 128], bf16, kind="ExternalInput")
    bg_d = nc.dram_tensor("Bg", [K, GS], bf16, kind="ExternalInput")
    ey_d = nc.dram_tensor("Ey", [128, B, NT, 5], bf16, kind="ExternalInput")
    out_d = nc.dram_tensor("OUT", [B, 3, 128, XCOLS], f32, kind="ExternalOutput")

    EXP = mybir.ActivationFunctionType.Exp

    with tile.TileContext(nc) as tc, ExitStack() as ctx:
        consts = ctx.enter_context(tc.tile_pool(name="consts", bufs=1))
        gram_pool = ctx.enter_context(tc.tile_pool(name="gram", bufs=2))
        mm1_pool = ctx.enter_context(tc.tile_pool(name="mm1", bufs=2, space="PSUM"))
        mm2_pool = ctx.enter_context(tc.tile_pool(name="mm2", bufs=2, space="PSUM"))
        small = ctx.enter_context(tc.tile_pool(name="small", bufs=4))
        outp = ctx.enter_context(tc.tile_pool(name="outp", bufs=1))

        a_sb = consts.tile([K, B, NT, 128], bf16)
        nc.sync.dma_start(out=a_sb, in_=ax_d[:])
        b_sb = consts.tile([K, GS], bf16)
        nc.sync.dma_start(out=b_sb, in_=bg_d[:])
        e_sb = consts.tile([128, B, NT, 5], bf16)
        nc.sync.dma_start(out=e_sb, in_=ey_d[:])
        out_sb = outp.tile([128, B, 3, XCOLS], f32)

        grams = [None] * B

        def emit_mm1_exp(b):
            gram = gram_pool.tile([128, SLOTS, 512], bf16, tag="gram")
            grams[b] = gram
            s = 0
            while s < SLOTS:
                k = min(3, SLOTS - s)
                ps = mm1_pool.tile([128, 3, 512], f32, tag="mm1")
                for i in range(k):
                    nt, j = divmod(s + i, JS)
                    nc.tensor.matmul(
                        ps[:, i, :],
                        a_sb[:, b, nt, :],
                        b_sb[:, j * 512:(j + 1) * 512],
                        start=True,
                        stop=True,
                    )
                nc.scalar.activation(
                    out=gram[:, s:s + k, :], in_=ps[:, 0:k, :], func=EXP
                )
                s += k

        def emit_mm2_norm(b):
            gram = grams[b]
            fm = mm2_pool.tile([128, XCOLS, 5], f32, tag="mm2")
            for gsub in range(XCOLS):
                j, r = divmod(gsub, 512 // 128)
                for nt in range(NT):
                    slot = nt * JS + j
                    nc.tensor.matmul(
                        fm[:, gsub, :],
                        gram[:, slot, r * 128:(r + 1) * 128],
                        e_sb[:, b, nt, :],
                        start=(nt == 0),
                        stop=(nt == NT - 1),
                    )
            fmc = small.tile([128, XCOLS, 5], f32, tag="fmc")
            nc.vector.tensor_copy(fmc, fm)
            recip = small.tile([128, XCOLS], f32, tag="recip")
            nc.vector.reciprocal(recip, fmc[:, :, 0])
            nc.vector.tensor_copy(out_sb[:, b, 0, :], fmc[:, :, 0])
            v1 = small.tile([128, XCOLS], f32, tag="v1")
            nc.vector.tensor_add(v1, fmc[:, :, 1], fmc[:, :, 3])
            nc.vector.tensor_mul(out_sb[:, b, 1, :], v1, recip)
            v2 = small.tile([128, XCOLS], f32, tag="v2")
            nc.vector.tensor_add(v2, fmc[:, :, 2], fmc[:, :, 4])
            nc.vector.tensor_mul(out_sb[:, b, 2, :], v2, recip)

        # software pipeline: mm1/exp of batch b overlaps mm2 of batch b-1
        for b in range(B + 1):
            if b < B:
                emit_mm1_exp(b)
            if b >= 1:
                emit_mm2_norm(b - 1)

        for b in range(B):
            for c in range(3):
                nc.sync.dma_start(out=out_d[b, c], in_=out_sb[:, b, c, :])

    nc.compile()
    return nc


def _split_hi_lo(a):
    import ml_dtypes

    bf = ml_dtypes.bfloat16
    hi = a.astype(bf).astype(np.float32)
    lo = (a - hi).astype(bf).astype(np.float32)
    return hi, lo


def _prepare_inputs(X, Y, grid):
    """Host-side packing: per-core input maps for the SPMD kernel."""
    import ml_dtypes

    bf = ml_dtypes.bfloat16
    X = np.asarray(X, np.float32)
    Y = np.asarray(Y, np.float32)
    grid = np.asarray(grid, np.float32)

    sx = -0.5 * np.sum(X * X, axis=-1)        # (B, N)
    sg = -0.5 * np.sum(grid * grid, axis=-1)  # (G,)
    xh, xl = _split_hi_lo(X)
    gh, gl = _split_hi_lo(grid)
    sxh, sxl = _split_hi_lo(sx)
    sgh, sgl = _split_hi_lo(sg)
    ones_n = np.ones((B, N), np.float32)
    ones_g = np.ones((G,), np.float32)

    # M'[n, g] = sum_k A[k, n] * Bm[k, g] = g.x - 0.5|x|^2 - 0.5|g|^2
    A = np.stack(
        [xh[..., 0], xh[..., 1], xl[..., 0], xl[..., 1],
         xh[..., 0], xh[..., 1], sxh, sxl, ones_n, ones_n],
        axis=1,
    )  # (B, K, N)
    Bm = np.stack(
        [gh[:, 0], gh[:, 1], gh[:, 0], gh[:, 1],
         gl[:, 0], gl[:, 1], ones_g, ones_g, sgh, sgl],
        axis=0,
    )  # (K, G)

    # Ax: [K, B, NT, 128]
    ax = A.transpose(1, 0, 2).reshape(K, B, NT, 128).astype(bf)
    # Ey: [128, B, NT, 5] = [1, yh0, yh1, yl0, yl1]
    yh, yl = _split_hi_lo(Y)
    E = np.stack([ones_n, yh[..., 0], yh[..., 1], yl[..., 0], yl[..., 1]], axis=-1)
    ey = E.reshape(B, NT, 128, 5).transpose(2, 0, 1, 3).astype(bf)
    ey = np.ascontiguousarray(ey)

    in_maps = []
    for c in range(NCORES):
        bg = np.ascontiguousarray(Bm[:, c * GS:(c + 1) * GS]).astype(bf)
        in_maps.append({"Ax": ax, "Bg": bg, "Ey": ey})
    return in_maps


def _run(in_maps, trace=False):
    from concourse.bass_utils import run_bass_kernel_spmd

    if "nc" not in _CACHE:
        _CACHE["nc"] = _build_nc()
    nc = _CACHE["nc"]
    return run_bass_kernel_spmd(nc, in_maps, core_ids=list(range(NCORES)),
                                trace=trace)


def kernel(X, Y, grid, _trace=False, _results_out=None):
    in_maps = _prepare_inputs(X, Y, grid)
    res = _run(in_maps, trace=_trace)
    out = np.empty((B, 3, 128, 128), np.float32)
    for c in range(NCORES):
        out[:, :, :, c * XCOLS:(c + 1) * XCOLS] = res.results[c]["OUT"]
    if _results_out is not None:
        _results_out.append(res)
    return out


# revision 9
# speedup vs baseline: 1.1523x; 1.1523x over previous
"""ConvCNP encoder kernel for 8 Trainium2 NeuronCores.

Computes, for full inputs X(4,1024,2), Y(4,1024,2), grid(16384,2):
    Gram = exp(-0.5*||grid-X||^2)          (B, G, n)
    FM   = Gram @ [1, Y]                   (B, G, 3)
    out  = [FM0, FM1/FM0, FM2/FM0] -> (B, 3, 128, 128)  (y, x image axes)

Sharding: grid axis G split 8 ways (2048 rows / core = 16 output
x-columns); every core handles all 4 batches; no cross-device reduction.

Per-core device pipeline:
  mm1 (PE):  -0.5*d2 as K=10 bf16 matmuls using a hi/lo split of
             g.x - 0.5|g|^2 - 0.5|x|^2 -> PSUM [n-tile 128, g 512].
             K=10 << 128, so 4 matmuls run concurrently in 32-row
             strips of the PE array via tile_position (inputs are
             replicated at partition offsets 0/32/64/96).
  exp (ACT): PSUM -> SBUF Gram (bf16), batched [128, <=4*512] exps
  mm2 (PE):  Gram[n,g128] stationary x E[n, (1,Yhi,Ylo)] moving,
             accumulated over 8 n-tiles -> FM [g(=y) 128, 5] PSUM
  norm (DVE): density reciprocal + multiplies, fp32
  DMA out:   [y, x] tiles per (b, c)
"""

import numpy as np

B = 4
N = 1024
G = 16384
NCORES = 8
GS = G // NCORES          # 2048 grid rows per core
NT = N // 128             # 8 context tiles
JS = GS // 512            # 4 g-blocks of 512 per core
K = 10                    # contraction rows of the d2 factorization
XCOLS = GS // 128         # 16 output x-columns per core

# combined input layout (free-dim offsets, bf16 elements)
A_W = B * 2 * 128         # 1024
B_W = GS                  # 2048
E_W = B * NT * 5          # 160
IN_W = A_W + B_W + E_W    # 3232

# exp slot-group sizes per half-batch (16 slots, j-outer / nt-inner)
GROUPS = (4, 3, 4, 3, 2)

_CACHE = {}


def _build_nc():
    import concourse.bacc as bacc
    import concourse.mybir as mybir
    import concourse.tile as tile
    from contextlib import ExitStack

    f32 = mybir.dt.float32
    bf16 = mybir.dt.bfloat16

    nc = bacc.Bacc("TRN2", target_bir_lowering=False, debug=False,
                   num_devices=NCORES)
    in_d = nc.dram_tensor("IN", [128, IN_W], bf16, kind="ExternalInput")
    out_d = nc.dram_tensor("OUT", [B, 3, 128, XCOLS], f32, kind="ExternalOutput")

    EXP = mybir.ActivationFunctionType.Exp

    with tile.TileContext(nc) as tc, ExitStack() as ctx:
        consts = ctx.enter_context(tc.tile_pool(name="consts", bufs=1))
        gram_pool = ctx.enter_context(tc.tile_pool(name="gram", bufs=4))
        mm1a_pool = ctx.enter_context(tc.tile_pool(name="mm1a", bufs=1, space="PSUM"))
        mm1b_pool = ctx.enter_context(tc.tile_pool(name="mm1b", bufs=1, space="PSUM"))
        mm2_pool = ctx.enter_context(tc.tile_pool(name="mm2", bufs=1, space="PSUM"))
        small = ctx.enter_context(tc.tile_pool(name="small", bufs=4))
        outp = ctx.enter_context(tc.tile_pool(name="outp", bufs=1))

        in_sb = consts.tile([128, IN_W], bf16)
        half = IN_W // 2
        nc.sync.dma_start(out=in_sb[:, 0:half], in_=in_d[:, 0:half])
        nc.gpsimd.dma_start(out=in_sb[:, half:IN_W], in_=in_d[:, half:IN_W])
        # views: A [128, B, 2, 128], Bg [128, GS], E [128, B, NT, 5]
        a_v = in_sb[:, 0:A_W].rearrange("p (b h m) -> p b h m", b=B, h=2)
        b_v = in_sb[:, A_W:A_W + B_W]
        e_v = in_sb[:, A_W + B_W:IN_W].rearrange("p (b t c) -> p b t c", b=B, t=NT)

        out_sb = outp.tile([128, B, 3, XCOLS], f32)
        grams = {}

        def emit_mm1_exp(b, h):
            gram = gram_pool.tile([128, 16, 512], bf16, tag="gram")
            grams[(b, h)] = gram
            s0 = 0
            for gi, gsz in enumerate(GROUPS):
                pool = (mm1a_pool, mm1b_pool)[gi % 2]
                cap = (4, 3)[gi % 2]
                ps = pool.tile([128, cap, 512], f32, tag=f"t{gi % 2}")
                for i in range(gsz):
                    s = s0 + i
                    j = 2 * h + s // 8
                    nt = s % 8
                    row = nt % 4
                    lhsT = a_v[32 * row:32 * row + K, b, nt // 4, :]
                    rhs = b_v[32 * row:32 * row + K, j * 512:(j + 1) * 512]
                    nc.tensor.matmul(ps[:, i, :], lhsT, rhs,
                                     start=True, stop=True,
                                     tile_position=(32 * row, 0))
                nc.scalar.activation(out=gram[:, s0:s0 + gsz, :],
                                     in_=ps[:, 0:gsz, :], func=EXP)
                s0 += gsz

        def emit_mm2(b, h):
            fm = grams[("fm", b)]
            gram = grams[(b, h)]
            for jj in range(2):
                j = 2 * h + jj
                for r in range(4):
                    gsub = j * 4 + r
                    for nt in range(NT):
                        nc.tensor.matmul(
                            fm[:, gsub, :],
                            gram[:, jj * 8 + nt, r * 128:(r + 1) * 128],
                            e_v[:, b, nt, :],
                            start=(nt == 0),
                            stop=(nt == NT - 1),
                        )

        def emit_norm(b):
            fm = grams[("fm", b)]
            fmc = small.tile([128, XCOLS, 5], f32, tag="fmc")
            nc.vector.tensor_copy(fmc, fm)
            recip = small.tile([128, XCOLS], f32, tag="recip")
            nc.vector.reciprocal(recip, fmc[:, :, 0])
            nc.vector.tensor_copy(out_sb[:, b, 0, :], fmc[:, :, 0])
            v1 = small.tile([128, XCOLS], f32, tag="v1")
            nc.vector.tensor_add(v1, fmc[:, :, 1], fmc[:, :, 3])
            nc.vector.tensor_mul(out_sb[:, b, 1, :], v1, recip)
            v2 = small.tile([128, XCOLS], f32, tag="v2")
            nc.vector.tensor_add(v2, fmc[:, :, 2], fmc[:, :, 4])
            nc.vector.tensor_mul(out_sb[:, b, 2, :], v2, recip)
            engines = (nc.sync, nc.gpsimd, nc.sync)
            for c in range(3):
                engines[c].dma_start(out=out_d[b, c], in_=out_sb[:, b, c, :])

        # software pipeline: mm1/exp of (b) overlaps mm2/norm of (b-1)
        for b in range(B):
            fm_t = mm2_pool.tile([128, XCOLS, 5], f32, tag="fm")
            grams[("fm", b)] = fm_t
            emit_mm1_exp(b, 0)
            emit_mm1_exp(b, 1)
            if b >= 1:
                emit_mm2(b - 1, 0)
                emit_mm2(b - 1, 1)
                emit_norm(b - 1)
        emit_mm2(B - 1, 0)
        emit_mm2(B - 1, 1)
        emit_norm(B - 1)

    nc.compile()
    return nc


def _split_hi_lo(a):
    import ml_dtypes

    bf = ml_dtypes.bfloat16
    hi = a.astype(bf).astype(np.float32)
    lo = (a - hi).astype(bf).astype(np.float32)
    return hi, lo


def _prepare_inputs(X, Y, grid):
    """Host-side packing: per-core input maps for the SPMD kernel."""
    import ml_dtypes

    bf = ml_dtypes.bfloat16
    X = np.asarray(X, np.float32)
    Y = np.asarray(Y, np.float32)
    grid = np.asarray(grid, np.float32)

    sx = -0.5 * np.sum(X * X, axis=-1)        # (B, N)
    sg = -0.5 * np.sum(grid * grid, axis=-1)  # (G,)
    xh, xl = _split_hi_lo(X)
    gh, gl = _split_hi_lo(grid)
    sxh, sxl = _split_hi_lo(sx)
    sgh, sgl = _split_hi_lo(sg)
    ones_n = np.ones((B, N), np.float32)
    ones_g = np.ones((G,), np.float32)

    # M'[n, g] = sum_k A[k, n] * Bm[k, g] = g.x - 0.5|x|^2 - 0.5|g|^2
    A = np.stack(
        [xh[..., 0], xh[..., 1], xl[..., 0], xl[..., 1],
         xh[..., 0], xh[..., 1], sxh, sxl, ones_n, ones_n],
        axis=1,
    )  # (B, K, N)
    Bm = np.stack(
        [gh[:, 0], gh[:, 1], gh[:, 0], gh[:, 1],
         gl[:, 0], gl[:, 1], ones_g, ones_g, sgh, sgl],
        axis=0,
    )  # (K, G)

    # A replicated into 4 row-strips: strip i (partitions 32i..32i+9)
    # holds A rows for nt = h*4 + i  -> [128, B, 2, 128]
    A4 = A.transpose(1, 0, 2).reshape(K, B, 2, 4, 128)  # k b h i p
    arep = np.zeros((128, B, 2, 128), np.float32)
    for i in range(4):
        arep[32 * i:32 * i + K] = A4[:, :, :, i, :]

    # E: [128, B, NT, 5] = [1, yh0, yh1, yl0, yl1]
    yh, yl = _split_hi_lo(Y)
    E = np.stack([ones_n, yh[..., 0], yh[..., 1], yl[..., 0], yl[..., 1]],
                 axis=-1)
    ey = E.reshape(B, NT, 128, 5).transpose(2, 0, 1, 3)

    in_maps = []
    for c in range(NCORES):
        # B replicated into the same 4 row-strips
        brep = np.zeros((128, GS), np.float32)
        for i in range(4):
            brep[32 * i:32 * i + K] = Bm[:, c * GS:(c + 1) * GS]
        packed = np.concatenate(
            [arep.reshape(128, A_W), brep, ey.reshape(128, E_W)], axis=1)
        in_maps.append({"IN": np.ascontiguousarray(packed).astype(bf)})
    return in_maps


def _run(in_maps, trace=False):
    from concourse.bass_utils import run_bass_kernel_spmd

    if "nc" not in _CACHE:
        _CACHE["nc"] = _build_nc()
    nc = _CACHE["nc"]
    return run_bass_kernel_spmd(nc, in_maps, core_ids=list(range(NCORES)),
                                trace=trace)


def kernel(X, Y, grid, _trace=False, _results_out=None):
    in_maps = _prepare_inputs(X, Y, grid)
    res = _run(in_maps, trace=_trace)
    out = np.empty((B, 3, 128, 128), np.float32)
    for c in range(NCORES):
        out[:, :, :, c * XCOLS:(c + 1) * XCOLS] = res.results[c]["OUT"]
    if _results_out is not None:
        _results_out.append(res)
    return out


# revision 13
# speedup vs baseline: 1.1843x; 1.0278x over previous
"""ConvCNP encoder kernel for 8 Trainium2 NeuronCores.

Computes, for full inputs X(4,1024,2), Y(4,1024,2), grid(16384,2):
    Gram = exp(-0.5*||grid-X||^2)          (B, G, n)
    FM   = Gram @ [1, Y]                   (B, G, 3)
    out  = [FM0, FM1/FM0, FM2/FM0] -> (B, 3, 128, 128)  (y, x image axes)

Sharding: grid axis G split 8 ways (2048 rows / core = 16 output
x-columns); every core handles all 4 batches; no cross-device reduction.

Per-core device pipeline:
  mm1 (PE):  -0.5*d2 as K=10 bf16 matmuls using a hi/lo split of
             g.x - 0.5|g|^2 - 0.5|x|^2 -> PSUM [n-tile 128, g 512].
             K=10 << 128, so 4 matmuls run concurrently in 32-row
             strips of the PE array via tile_position (inputs are
             replicated at partition offsets 0/32/64/96).
  exp (ACT): PSUM -> SBUF Gram (bf16), batched [128, <=4*512] exps
  mm2 (PE):  Gram[n,g128] stationary x E[n, (1,Yhi,Ylo)] moving,
             accumulated over 8 n-tiles -> FM [g(=y) 128, 5] PSUM
  norm (DVE): density reciprocal + multiplies, fp32
  DMA out:   [y, x] tiles per (b, c)
"""

import numpy as np

B = 4
N = 1024
G = 16384
NCORES = 8
GS = G // NCORES          # 2048 grid rows per core
NT = N // 128             # 8 context tiles
JS = GS // 512            # 4 g-blocks of 512 per core
K = 10                    # contraction rows of the d2 factorization
XCOLS = GS // 128         # 16 output x-columns per core

# combined input layout (free-dim offsets, bf16 elements)
A_W = B * 2 * 128         # 1024
B_W = GS                  # 2048
E_W = B * NT * 5          # 160
IN_W = A_W + B_W + E_W    # 3232

_CACHE = {}


def _build_nc():
    import concourse.bacc as bacc
    import concourse.mybir as mybir
    import concourse.tile as tile
    from contextlib import ExitStack

    f32 = mybir.dt.float32
    bf16 = mybir.dt.bfloat16

    nc = bacc.Bacc("TRN2", target_bir_lowering=False, debug=False,
                   num_devices=NCORES)
    in_d = nc.dram_tensor("IN", [128, IN_W], bf16, kind="ExternalInput")
    out_d = nc.dram_tensor("OUT", [B, 3, 128, XCOLS], f32, kind="ExternalOutput")

    EXP = mybir.ActivationFunctionType.Exp

    with tile.TileContext(nc) as tc, ExitStack() as ctx:
        consts = ctx.enter_context(tc.tile_pool(name="consts", bufs=1))
        gram_pool = ctx.enter_context(tc.tile_pool(name="gram", bufs=4))
        mm1a_pool = ctx.enter_context(tc.tile_pool(name="mm1a", bufs=1, space="PSUM"))
        mm1b_pool = ctx.enter_context(tc.tile_pool(name="mm1b", bufs=1, space="PSUM"))
        mm2_pool = ctx.enter_context(tc.tile_pool(name="mm2", bufs=1, space="PSUM"))
        small = ctx.enter_context(tc.tile_pool(name="small", bufs=4))
        outp = ctx.enter_context(tc.tile_pool(name="outp", bufs=1))

        in_sb = consts.tile([128, IN_W], bf16)
        # first mm1 group needs A + B[:, :1024]; land those first, in
        # parallel on two queues, with the rest following on sync.
        c1 = A_W + 1024
        nc.sync.dma_start(out=in_sb[:, 0:A_W], in_=in_d[:, 0:A_W])
        nc.gpsimd.dma_start(out=in_sb[:, A_W:c1], in_=in_d[:, A_W:c1])
        nc.sync.dma_start(out=in_sb[:, c1:IN_W], in_=in_d[:, c1:IN_W])
        # views: A [128, B, 2, 128], Bg [128, GS], E [128, B, NT, 5]
        a_v = in_sb[:, 0:A_W].rearrange("p (b h m) -> p b h m", b=B, h=2)
        b_v = in_sb[:, A_W:A_W + B_W]
        e_v = in_sb[:, A_W + B_W:IN_W].rearrange("p (b t c) -> p b t c", b=B, t=NT)

        out_sb = outp.tile([128, B, 3, XCOLS], f32)
        grams = {}

        def emit_mm1_exp(b, h):
            gram = gram_pool.tile([128, 16, 512], bf16, tag="gram")
            grams[(b, h)] = gram
            # alternate which psum tag starts each half so consecutive
            # same-tag allocations never sit at a half boundary (that
            # would stall ACT for the refill time)
            flip = (2 * b + h) % 2
            sizes = (4, 3, 4, 3, 2) if flip == 0 else (3, 4, 3, 4, 2)
            s0 = 0
            for gi, gsz in enumerate(sizes):
                sel = (gi + flip) % 2
                pool = (mm1a_pool, mm1b_pool)[sel]
                cap = (4, 3)[sel]
                ps = pool.tile([128, cap, 512], f32, tag=f"t{sel}")
                for i in range(gsz):
                    s = s0 + i
                    j = 2 * h + s // 8
                    nt = s % 8
                    row = nt % 4
                    lhsT = a_v[32 * row:32 * row + K, b, nt // 4, :]
                    rhs = b_v[32 * row:32 * row + K, j * 512:(j + 1) * 512]
                    nc.tensor.matmul(ps[:, i, :], lhsT, rhs,
                                     start=True, stop=True,
                                     tile_position=(32 * row, 0))
                nc.scalar.activation(out=gram[:, s0:s0 + gsz, :],
                                     in_=ps[:, 0:gsz, :], func=EXP)
                s0 += gsz

        def emit_mm2(b, h):
            fm = grams[("fm", b)]
            gram = grams[(b, h)]
            for jj in range(2):
                j = 2 * h + jj
                for r in range(4):
                    gsub = j * 4 + r
                    for nt in range(NT):
                        nc.tensor.matmul(
                            fm[:, gsub, :],
                            gram[:, jj * 8 + nt, r * 128:(r + 1) * 128],
                            e_v[:, b, nt, :],
                            start=(nt == 0),
                            stop=(nt == NT - 1),
                        )

        def emit_norm(b, h):
            fm = grams[("fm", b)]
            g0 = 8 * h
            sl = slice(g0, g0 + 8)
            fmc = small.tile([128, 8, 5], f32, tag="fmc")
            nc.vector.tensor_copy(fmc, fm[:, sl, :])
            recip = small.tile([128, 8], f32, tag="recip")
            nc.vector.reciprocal(recip, fmc[:, :, 0])
            nc.vector.tensor_copy(out_sb[:, b, 0, sl], fmc[:, :, 0])
            v1 = small.tile([128, 8], f32, tag="v1")
            nc.vector.tensor_add(v1, fmc[:, :, 1], fmc[:, :, 3])
            nc.vector.tensor_mul(out_sb[:, b, 1, sl], v1, recip)
            v2 = small.tile([128, 8], f32, tag="v2")
            nc.vector.tensor_add(v2, fmc[:, :, 2], fmc[:, :, 4])
            nc.vector.tensor_mul(out_sb[:, b, 2, sl], v2, recip)
            engines = (nc.sync, nc.gpsimd, nc.sync)
            for c in range(3):
                engines[c].dma_start(out=out_d[b, c, :, sl],
                                     in_=out_sb[:, b, c, sl])

        # software pipeline: mm1/exp of (b) overlaps mm2/norm of (b-1)
        for b in range(B):
            fm_t = mm2_pool.tile([128, XCOLS, 5], f32, tag="fm")
            grams[("fm", b)] = fm_t
            emit_mm1_exp(b, 0)
            emit_mm1_exp(b, 1)
            if b >= 1:
                for h in range(2):
                    emit_mm2(b - 1, h)
                    emit_norm(b - 1, h)
        for h in range(2):
            emit_mm2(B - 1, h)
            emit_norm(B - 1, h)

    nc.compile()
    return nc


def _split_hi_lo(a):
    import ml_dtypes

    bf = ml_dtypes.bfloat16
    hi = a.astype(bf).astype(np.float32)
    lo = (a - hi).astype(bf).astype(np.float32)
    return hi, lo


def _prepare_inputs(X, Y, grid):
    """Host-side packing: per-core input maps for the SPMD kernel."""
    import ml_dtypes

    bf = ml_dtypes.bfloat16
    X = np.asarray(X, np.float32)
    Y = np.asarray(Y, np.float32)
    grid = np.asarray(grid, np.float32)

    sx = -0.5 * np.sum(X * X, axis=-1)        # (B, N)
    sg = -0.5 * np.sum(grid * grid, axis=-1)  # (G,)
    xh, xl = _split_hi_lo(X)
    gh, gl = _split_hi_lo(grid)
    sxh, sxl = _split_hi_lo(sx)
    sgh, sgl = _split_hi_lo(sg)
    ones_n = np.ones((B, N), np.float32)
    ones_g = np.ones((G,), np.float32)

    # M'[n, g] = sum_k A[k, n] * Bm[k, g] = g.x - 0.5|x|^2 - 0.5|g|^2
    A = np.stack(
        [xh[..., 0], xh[..., 1], xl[..., 0], xl[..., 1],
         xh[..., 0], xh[..., 1], sxh, sxl, ones_n, ones_n],
        axis=1,
    )  # (B, K, N)
    Bm = np.stack(
        [gh[:, 0], gh[:, 1], gh[:, 0], gh[:, 1],
         gl[:, 0], gl[:, 1], ones_g, ones_g, sgh, sgl],
        axis=0,
    )  # (K, G)

    # A replicated into 4 row-strips: strip i (partitions 32i..32i+9)
    # holds A rows for nt = h*4 + i  -> [128, B, 2, 128]
    A4 = A.transpose(1, 0, 2).reshape(K, B, 2, 4, 128)  # k b h i p
    arep = np.zeros((128, B, 2, 128), np.float32)
    for i in range(4):
        arep[32 * i:32 * i + K] = A4[:, :, :, i, :]

    # E: [128, B, NT, 5] = [1, yh0, yh1, yl0, yl1]
    yh, yl = _split_hi_lo(Y)
    E = np.stack([ones_n, yh[..., 0], yh[..., 1], yl[..., 0], yl[..., 1]],
                 axis=-1)
    ey = E.reshape(B, NT, 128, 5).transpose(2, 0, 1, 3)

    in_maps = []
    for c in range(NCORES):
        # B replicated into the same 4 row-strips
        brep = np.zeros((128, GS), np.float32)
        for i in range(4):
            brep[32 * i:32 * i + K] = Bm[:, c * GS:(c + 1) * GS]
        packed = np.concatenate(
            [arep.reshape(128, A_W), brep, ey.reshape(128, E_W)], axis=1)
        in_maps.append({"IN": np.ascontiguousarray(packed).astype(bf)})
    return in_maps


def _run(in_maps, trace=False):
    from concourse.bass_utils import run_bass_kernel_spmd

    if "nc" not in _CACHE:
        _CACHE["nc"] = _build_nc()
    nc = _CACHE["nc"]
    return run_bass_kernel_spmd(nc, in_maps, core_ids=list(range(NCORES)),
                                trace=trace)


def kernel(X, Y, grid, _trace=False, _results_out=None):
    in_maps = _prepare_inputs(X, Y, grid)
    res = _run(in_maps, trace=_trace)
    out = np.empty((B, 3, 128, 128), np.float32)
    for c in range(NCORES):
        out[:, :, :, c * XCOLS:(c + 1) * XCOLS] = res.results[c]["OUT"]
    if _results_out is not None:
        _results_out.append(res)
    return out


# revision 15
# speedup vs baseline: 1.2287x; 1.0375x over previous
"""ConvCNP encoder kernel for 8 Trainium2 NeuronCores.

Computes, for full inputs X(4,1024,2), Y(4,1024,2), grid(16384,2):
    Gram = exp(-0.5*||grid-X||^2)          (B, G, n)
    FM   = Gram @ [1, Y]                   (B, G, 3)
    out  = [FM0, FM1/FM0, FM2/FM0] -> (B, 3, 128, 128)  (y, x image axes)

Sharding: grid axis G split 8 ways (2048 rows / core = 16 output
x-columns); every core handles all 4 batches; no cross-device reduction.

Per-core device pipeline:
  mm1 (PE):  -0.5*d2 as K=10 bf16 matmuls using a hi/lo split of
             g.x - 0.5|g|^2 - 0.5|x|^2 -> PSUM [n-tile 128, g 512].
             K=10 << 128, so 4 matmuls run concurrently in 32-row
             strips of the PE array via tile_position (inputs are
             replicated at partition offsets 0/32/64/96).
  exp (ACT): PSUM -> SBUF Gram (bf16), batched [128, <=4*512] exps
  mm2 (PE):  Gram[n,g128] stationary x E[n, (1,Yhi,Ylo)] moving,
             accumulated over 8 n-tiles -> FM [g(=y) 128, 5] PSUM
  norm (DVE): density reciprocal + multiplies, fp32
  DMA out:   [y, x] tiles per (b, c)
"""

import numpy as np

B = 4
N = 1024
G = 16384
NCORES = 8
GS = G // NCORES          # 2048 grid rows per core
NT = N // 128             # 8 context tiles
JS = GS // 512            # 4 g-blocks of 512 per core
K = 10                    # contraction rows of the d2 factorization
XCOLS = GS // 128         # 16 output x-columns per core

# combined input layout (free-dim offsets, bf16 elements)
A_W = B * 2 * 128         # 1024
B_W = GS                  # 2048
E_W = B * NT * 5          # 160
IN_W = A_W + B_W + E_W    # 3232

_CACHE = {}


def _build_nc():
    import concourse.bacc as bacc
    import concourse.mybir as mybir
    import concourse.tile as tile
    from contextlib import ExitStack

    f32 = mybir.dt.float32
    bf16 = mybir.dt.bfloat16

    nc = bacc.Bacc("TRN2", target_bir_lowering=False, debug=False,
                   num_devices=NCORES)
    in_d = nc.dram_tensor("IN", [128, IN_W], bf16, kind="ExternalInput")
    out_d = nc.dram_tensor("OUT", [B, 3, 128, XCOLS], f32, kind="ExternalOutput")

    EXP = mybir.ActivationFunctionType.Exp

    with tile.TileContext(nc) as tc, ExitStack() as ctx:
        consts = ctx.enter_context(tc.tile_pool(name="consts", bufs=1))
        gram_pool = ctx.enter_context(tc.tile_pool(name="gram", bufs=4))
        mm1a_pool = ctx.enter_context(tc.tile_pool(name="mm1a", bufs=1, space="PSUM"))
        mm1b_pool = ctx.enter_context(tc.tile_pool(name="mm1b", bufs=1, space="PSUM"))
        mm2_pool = ctx.enter_context(tc.tile_pool(name="mm2", bufs=1, space="PSUM"))
        small = ctx.enter_context(tc.tile_pool(name="small", bufs=4))
        outp = ctx.enter_context(tc.tile_pool(name="outp", bufs=1))

        # separate tiles so consumers only wait for the DMA they need;
        # first mm1 group needs A + B[:, :1024] — those land first on
        # two parallel queues.
        a_sb = consts.tile([128, A_W], bf16)
        b_lo = consts.tile([128, B_W // 2], bf16)
        b_hi = consts.tile([128, B_W // 2], bf16)
        e_sb = consts.tile([128, E_W], bf16)
        c1 = A_W + B_W // 2
        c2 = A_W + B_W
        nc.sync.dma_start(out=a_sb, in_=in_d[:, 0:A_W])
        nc.gpsimd.dma_start(out=b_lo, in_=in_d[:, A_W:c1])
        nc.sync.dma_start(out=b_hi, in_=in_d[:, c1:c2])
        nc.gpsimd.dma_start(out=e_sb, in_=in_d[:, c2:IN_W])
        # views: A [128, B, 2, 128], Bg lo/hi [128, 1024], E [128, B, NT, 5]
        a_v = a_sb.rearrange("p (b h m) -> p b h m", b=B, h=2)
        e_v = e_sb.rearrange("p (b t c) -> p b t c", b=B, t=NT)

        def b_slice(j):
            t = (b_lo, b_hi)[j // 2]
            jj = j % 2
            return t[:, jj * 512:(jj + 1) * 512]

        out_sb = outp.tile([128, B, 3, XCOLS], f32)
        grams = {}

        def emit_mm1_exp(b, h):
            gram = gram_pool.tile([128, 16, 512], bf16, tag="gram")
            grams[(b, h)] = gram
            # alternate which psum tag starts each half so consecutive
            # same-tag allocations never sit at a half boundary (that
            # would stall ACT for the refill time)
            flip = (2 * b + h) % 2
            sizes = (4, 3, 4, 3, 2) if flip == 0 else (3, 4, 3, 4, 2)
            s0 = 0
            for gi, gsz in enumerate(sizes):
                sel = (gi + flip) % 2
                pool = (mm1a_pool, mm1b_pool)[sel]
                cap = (4, 3)[sel]
                ps = pool.tile([128, cap, 512], f32, tag=f"t{sel}")
                for i in range(gsz):
                    s = s0 + i
                    j = 2 * h + s // 8
                    nt = s % 8
                    row = nt % 4
                    lhsT = a_v[32 * row:32 * row + K, b, nt // 4, :]
                    rhs = b_slice(j)[32 * row:32 * row + K, :]
                    nc.tensor.matmul(ps[:, i, :], lhsT, rhs,
                                     start=True, stop=True,
                                     tile_position=(32 * row, 0))
                nc.scalar.activation(out=gram[:, s0:s0 + gsz, :],
                                     in_=ps[:, 0:gsz, :], func=EXP)
                s0 += gsz

        def emit_mm2(b, h):
            fm = grams[("fm", b)]
            gram = grams[(b, h)]
            for jj in range(2):
                j = 2 * h + jj
                for r in range(4):
                    gsub = j * 4 + r
                    for nt in range(NT):
                        nc.tensor.matmul(
                            fm[:, gsub, :],
                            gram[:, jj * 8 + nt, r * 128:(r + 1) * 128],
                            e_v[:, b, nt, :],
                            start=(nt == 0),
                            stop=(nt == NT - 1),
                        )

        def emit_norm(b, h):
            fm = grams[("fm", b)]
            g0 = 8 * h
            sl = slice(g0, g0 + 8)
            fmc = small.tile([128, 8, 5], f32, tag="fmc")
            nc.vector.tensor_copy(fmc, fm[:, sl, :])
            recip = small.tile([128, 8], f32, tag="recip")
            nc.vector.reciprocal(recip, fmc[:, :, 0])
            nc.vector.tensor_copy(out_sb[:, b, 0, sl], fmc[:, :, 0])
            v1 = small.tile([128, 8], f32, tag="v1")
            nc.vector.tensor_add(v1, fmc[:, :, 1], fmc[:, :, 3])
            nc.vector.tensor_mul(out_sb[:, b, 1, sl], v1, recip)
            v2 = small.tile([128, 8], f32, tag="v2")
            nc.vector.tensor_add(v2, fmc[:, :, 2], fmc[:, :, 4])
            nc.vector.tensor_mul(out_sb[:, b, 2, sl], v2, recip)
            engines = (nc.sync, nc.gpsimd, nc.sync)
            for c in range(3):
                engines[c].dma_start(out=out_d[b, c, :, sl],
                                     in_=out_sb[:, b, c, sl])

        # software pipeline: mm1/exp of (b) overlaps mm2/norm of (b-1)
        for b in range(B):
            fm_t = mm2_pool.tile([128, XCOLS, 5], f32, tag="fm")
            grams[("fm", b)] = fm_t
            emit_mm1_exp(b, 0)
            emit_mm1_exp(b, 1)
            if b >= 1:
                for h in range(2):
                    emit_mm2(b - 1, h)
                    emit_norm(b - 1, h)
        for h in range(2):
            emit_mm2(B - 1, h)
            emit_norm(B - 1, h)

    nc.compile()
    return nc


def _split_hi_lo(a):
    import ml_dtypes

    bf = ml_dtypes.bfloat16
    hi = a.astype(bf).astype(np.float32)
    lo = (a - hi).astype(bf).astype(np.float32)
    return hi, lo


def _prepare_inputs(X, Y, grid):
    """Host-side packing: per-core input maps for the SPMD kernel."""
    import ml_dtypes

    bf = ml_dtypes.bfloat16
    X = np.asarray(X, np.float32)
    Y = np.asarray(Y, np.float32)
    grid = np.asarray(grid, np.float32)

    sx = -0.5 * np.sum(X * X, axis=-1)        # (B, N)
    sg = -0.5 * np.sum(grid * grid, axis=-1)  # (G,)
    xh, xl = _split_hi_lo(X)
    gh, gl = _split_hi_lo(grid)
    sxh, sxl = _split_hi_lo(sx)
    sgh, sgl = _split_hi_lo(sg)
    ones_n = np.ones((B, N), np.float32)
    ones_g = np.ones((G,), np.float32)

    # M'[n, g] = sum_k A[k, n] * Bm[k, g] = g.x - 0.5|x|^2 - 0.5|g|^2
    A = np.stack(
        [xh[..., 0], xh[..., 1], xl[..., 0], xl[..., 1],
         xh[..., 0], xh[..., 1], sxh, sxl, ones_n, ones_n],
        axis=1,
    )  # (B, K, N)
    Bm = np.stack(
        [gh[:, 0], gh[:, 1], gh[:, 0], gh[:, 1],
         gl[:, 0], gl[:, 1], ones_g, ones_g, sgh, sgl],
        axis=0,
    )  # (K, G)

    # A replicated into 4 row-strips: strip i (partitions 32i..32i+9)
    # holds A rows for nt = h*4 + i  -> [128, B, 2, 128]
    A4 = A.transpose(1, 0, 2).reshape(K, B, 2, 4, 128)  # k b h i p
    arep = np.zeros((128, B, 2, 128), np.float32)
    for i in range(4):
        arep[32 * i:32 * i + K] = A4[:, :, :, i, :]

    # E: [128, B, NT, 5] = [1, yh0, yh1, yl0, yl1]
    yh, yl = _split_hi_lo(Y)
    E = np.stack([ones_n, yh[..., 0], yh[..., 1], yl[..., 0], yl[..., 1]],
                 axis=-1)
    ey = E.reshape(B, NT, 128, 5).transpose(2, 0, 1, 3)

    in_maps = []
    for c in range(NCORES):
        # B replicated into the same 4 row-strips
        brep = np.zeros((128, GS), np.float32)
        for i in range(4):
            brep[32 * i:32 * i + K] = Bm[:, c * GS:(c + 1) * GS]
        packed = np.concatenate(
            [arep.reshape(128, A_W), brep, ey.reshape(128, E_W)], axis=1)
        in_maps.append({"IN": np.ascontiguousarray(packed).astype(bf)})
    return in_maps


def _run(in_maps, trace=False):
    from concourse.bass_utils import run_bass_kernel_spmd

    if "nc" not in _CACHE:
        _CACHE["nc"] = _build_nc()
    nc = _CACHE["nc"]
    return run_bass_kernel_spmd(nc, in_maps, core_ids=list(range(NCORES)),
                                trace=trace)


def kernel(X, Y, grid, _trace=False, _results_out=None):
    in_maps = _prepare_inputs(X, Y, grid)
    res = _run(in_maps, trace=_trace)
    out = np.empty((B, 3, 128, 128), np.float32)
    for c in range(NCORES):
        out[:, :, :, c * XCOLS:(c + 1) * XCOLS] = res.results[c]["OUT"]
    if _results_out is not None:
        _results_out.append(res)
    return out


# revision 18
# speedup vs baseline: 1.2513x; 1.0184x over previous
"""ConvCNP encoder kernel for 8 Trainium2 NeuronCores.

Computes, for full inputs X(4,1024,2), Y(4,1024,2), grid(16384,2):
    Gram = exp(-0.5*||grid-X||^2)          (B, G, n)
    FM   = Gram @ [1, Y]                   (B, G, 3)
    out  = [FM0, FM1/FM0, FM2/FM0] -> (B, 3, 128, 128)  (y, x image axes)

Sharding: grid axis G split 8 ways (2048 rows / core = 16 output
x-columns); every core handles all 4 batches; no cross-device reduction.

Per-core device pipeline:
  mm1 (PE):  -0.5*d2 as K=10 bf16 matmuls using a hi/lo split of
             g.x - 0.5|g|^2 - 0.5|x|^2 -> PSUM [n-tile 128, g 512].
             K=10 << 128, so 4 matmuls run concurrently in 32-row
             strips of the PE array via tile_position (inputs are
             replicated at partition offsets 0/32/64/96).
  exp (ACT): PSUM -> SBUF Gram (bf16), batched [128, <=4*512] exps
  mm2 (PE):  Gram[n,g128] stationary x E[n, (1,Yhi,Ylo)] moving,
             accumulated over 8 n-tiles -> FM [g(=y) 128, 5] PSUM
  norm (DVE): density reciprocal + multiplies, fp32
  DMA out:   [y, x] tiles per (b, c)
"""

import numpy as np

B = 4
N = 1024
G = 16384
NCORES = 8
GS = G // NCORES          # 2048 grid rows per core
NT = N // 128             # 8 context tiles
JS = GS // 512            # 4 g-blocks of 512 per core
K = 10                    # contraction rows of the d2 factorization
XCOLS = GS // 128         # 16 output x-columns per core

# combined input layout (free-dim offsets, bf16 elements)
A_W = B * 2 * 128         # 1024
B_W = GS                  # 2048
E_W = B * NT * 5          # 160
IN_W = A_W + B_W + E_W    # 3232

_CACHE = {}


def _build_nc():
    import concourse.bacc as bacc
    import concourse.mybir as mybir
    import concourse.tile as tile
    from contextlib import ExitStack

    f32 = mybir.dt.float32
    bf16 = mybir.dt.bfloat16

    nc = bacc.Bacc("TRN2", target_bir_lowering=False, debug=False,
                   num_devices=NCORES)
    in_d = nc.dram_tensor("IN", [128, IN_W], bf16, kind="ExternalInput")
    out_d = nc.dram_tensor("OUT", [B, 3, 128, XCOLS], f32, kind="ExternalOutput")

    EXP = mybir.ActivationFunctionType.Exp

    with tile.TileContext(nc) as tc, ExitStack() as ctx:
        consts = ctx.enter_context(tc.tile_pool(name="consts", bufs=1))
        gram_pool = ctx.enter_context(tc.tile_pool(name="gram", bufs=4))
        mm1a_pool = ctx.enter_context(tc.tile_pool(name="mm1a", bufs=1, space="PSUM"))
        mm1b_pool = ctx.enter_context(tc.tile_pool(name="mm1b", bufs=1, space="PSUM"))
        mm2_pool = ctx.enter_context(tc.tile_pool(name="mm2", bufs=1, space="PSUM"))
        small = ctx.enter_context(tc.tile_pool(name="small", bufs=4))
        outp = ctx.enter_context(tc.tile_pool(name="outp", bufs=1))

        # separate tiles so consumers only wait for the DMA they need;
        # the first mm1 group touches only A[b0] and B[j0] — land those
        # first, one on each queue, before the bulk.
        a0_sb = consts.tile([128, 2 * 128], bf16)
        a123_sb = consts.tile([128, 3 * 2 * 128], bf16)
        b_t = [consts.tile([128, 512], bf16, name=f"bj{j}", tag=f"bj{j}")
               for j in range(JS)]
        e_sb = consts.tile([128, E_W], bf16)

        def in_col(c0, w):
            return in_d[:, c0:c0 + w]

        nc.sync.dma_start(out=b_t[0], in_=in_col(A_W, 512))
        nc.gpsimd.dma_start(out=a0_sb, in_=in_col(0, 256))
        nc.gpsimd.dma_start(out=b_t[1], in_=in_col(A_W + 512, 512))
        nc.sync.dma_start(out=a123_sb, in_=in_col(256, 768))
        nc.sync.dma_start(out=b_t[2], in_=in_col(A_W + 1024, 512))
        nc.gpsimd.dma_start(out=b_t[3], in_=in_col(A_W + 1536, 512))
        nc.sync.dma_start(out=e_sb, in_=in_col(A_W + B_W, E_W))

        # views: A[b] -> [128, 2, 128], E [128, B, NT, 5]
        a0_v = a0_sb.rearrange("p (h m) -> p h m", h=2)
        a123_v = a123_sb.rearrange("p (b h m) -> p b h m", b=3, h=2)
        e_v = e_sb.rearrange("p (b t c) -> p b t c", b=B, t=NT)

        def a_slice(b, row, h4):
            if b == 0:
                return a0_v[32 * row:32 * row + K, h4, :]
            return a123_v[32 * row:32 * row + K, b - 1, h4, :]

        def b_slice(j):
            return b_t[j]

        out_sb = outp.tile([128, B, 3, XCOLS], f32)
        grams = {}

        def emit_mm1_exp(b, h):
            gram = gram_pool.tile([128, 16, 512], bf16, tag="gram")
            grams[(b, h)] = gram
            # alternate which psum tag starts each half so consecutive
            # same-tag allocations never sit at a half boundary (that
            # would stall ACT for the refill time)
            flip = (2 * b + h) % 2
            sizes = (4, 3, 4, 3, 2) if flip == 0 else (3, 4, 3, 4, 2)
            s0 = 0
            for gi, gsz in enumerate(sizes):
                sel = (gi + flip) % 2
                pool = (mm1a_pool, mm1b_pool)[sel]
                cap = (4, 3)[sel]
                ps = pool.tile([128, cap, 512], f32, tag=f"t{sel}")
                for i in range(gsz):
                    s = s0 + i
                    j = 2 * h + s // 8
                    nt = s % 8
                    row = nt % 4
                    lhsT = a_slice(b, row, nt // 4)
                    rhs = b_slice(j)[32 * row:32 * row + K, :]
                    nc.tensor.matmul(ps[:, i, :], lhsT, rhs,
                                     start=True, stop=True,
                                     tile_position=(32 * row, 0))
                nc.scalar.activation(out=gram[:, s0:s0 + gsz, :],
                                     in_=ps[:, 0:gsz, :], func=EXP)
                s0 += gsz

        def emit_mm2(b, h):
            fm = grams[("fm", b)]
            gram = grams[(b, h)]
            for jj in range(2):
                j = 2 * h + jj
                for r in range(4):
                    gsub = j * 4 + r
                    for nt in range(NT):
                        nc.tensor.matmul(
                            fm[:, gsub, :],
                            gram[:, jj * 8 + nt, r * 128:(r + 1) * 128],
                            e_v[:, b, nt, :],
                            start=(nt == 0),
                            stop=(nt == NT - 1),
                        )

        def emit_norm(b, h):
            fm = grams[("fm", b)]
            g0 = 8 * h
            sl = slice(g0, g0 + 8)
            fmc = small.tile([128, 8, 5], f32, tag="fmc")
            nc.vector.tensor_copy(fmc, fm[:, sl, :])
            recip = small.tile([128, 8], f32, tag="recip")
            nc.vector.reciprocal(recip, fmc[:, :, 0])
            nc.vector.tensor_copy(out_sb[:, b, 0, sl], fmc[:, :, 0])
            v1 = small.tile([128, 8], f32, tag="v1")
            nc.vector.tensor_add(v1, fmc[:, :, 1], fmc[:, :, 3])
            nc.vector.tensor_mul(out_sb[:, b, 1, sl], v1, recip)
            v2 = small.tile([128, 8], f32, tag="v2")
            nc.vector.tensor_add(v2, fmc[:, :, 2], fmc[:, :, 4])
            nc.vector.tensor_mul(out_sb[:, b, 2, sl], v2, recip)
            engines = (nc.sync, nc.gpsimd, nc.sync)
            for c in range(3):
                engines[c].dma_start(out=out_d[b, c, :, sl],
                                     in_=out_sb[:, b, c, sl])

        # software pipeline: mm1/exp of (b) overlaps mm2/norm of (b-1)
        for b in range(B):
            fm_t = mm2_pool.tile([128, XCOLS, 5], f32, tag="fm")
            grams[("fm", b)] = fm_t
            emit_mm1_exp(b, 0)
            emit_mm1_exp(b, 1)
            if b >= 1:
                for h in range(2):
                    emit_mm2(b - 1, h)
                    emit_norm(b - 1, h)
        for h in range(2):
            emit_mm2(B - 1, h)
            emit_norm(B - 1, h)

    nc.compile()
    return nc


def _split_hi_lo(a):
    import ml_dtypes

    bf = ml_dtypes.bfloat16
    hi = a.astype(bf).astype(np.float32)
    lo = (a - hi).astype(bf).astype(np.float32)
    return hi, lo


def _prepare_inputs(X, Y, grid):
    """Host-side packing: per-core input maps for the SPMD kernel."""
    import ml_dtypes

    bf = ml_dtypes.bfloat16
    X = np.asarray(X, np.float32)
    Y = np.asarray(Y, np.float32)
    grid = np.asarray(grid, np.float32)

    sx = -0.5 * np.sum(X * X, axis=-1)        # (B, N)
    sg = -0.5 * np.sum(grid * grid, axis=-1)  # (G,)
    xh, xl = _split_hi_lo(X)
    gh, gl = _split_hi_lo(grid)
    sxh, sxl = _split_hi_lo(sx)
    sgh, sgl = _split_hi_lo(sg)
    ones_n = np.ones((B, N), np.float32)
    ones_g = np.ones((G,), np.float32)

    # M'[n, g] = sum_k A[k, n] * Bm[k, g] = g.x - 0.5|x|^2 - 0.5|g|^2
    A = np.stack(
        [xh[..., 0], xh[..., 1], xl[..., 0], xl[..., 1],
         xh[..., 0], xh[..., 1], sxh, sxl, ones_n, ones_n],
        axis=1,
    )  # (B, K, N)
    Bm = np.stack(
        [gh[:, 0], gh[:, 1], gh[:, 0], gh[:, 1],
         gl[:, 0], gl[:, 1], ones_g, ones_g, sgh, sgl],
        axis=0,
    )  # (K, G)

    # A replicated into 4 row-strips: strip i (partitions 32i..32i+9)
    # holds A rows for nt = h*4 + i  -> [128, B, 2, 128]
    A4 = A.transpose(1, 0, 2).reshape(K, B, 2, 4, 128)  # k b h i p
    arep = np.zeros((128, B, 2, 128), np.float32)
    for i in range(4):
        arep[32 * i:32 * i + K] = A4[:, :, :, i, :]

    # E: [128, B, NT, 5] = [1, yh0, yh1, yl0, yl1]
    yh, yl = _split_hi_lo(Y)
    E = np.stack([ones_n, yh[..., 0], yh[..., 1], yl[..., 0], yl[..., 1]],
                 axis=-1)
    ey = E.reshape(B, NT, 128, 5).transpose(2, 0, 1, 3)

    in_maps = []
    for c in range(NCORES):
        # B replicated into the same 4 row-strips
        brep = np.zeros((128, GS), np.float32)
        for i in range(4):
            brep[32 * i:32 * i + K] = Bm[:, c * GS:(c + 1) * GS]
        packed = np.concatenate(
            [arep.reshape(128, A_W), brep, ey.reshape(128, E_W)], axis=1)
        in_maps.append({"IN": np.ascontiguousarray(packed).astype(bf)})
    return in_maps


def _run(in_maps, trace=False):
    from concourse.bass_utils import run_bass_kernel_spmd

    if "nc" not in _CACHE:
        _CACHE["nc"] = _build_nc()
    nc = _CACHE["nc"]
    return run_bass_kernel_spmd(nc, in_maps, core_ids=list(range(NCORES)),
                                trace=trace)


def kernel(X, Y, grid, _trace=False, _results_out=None):
    in_maps = _prepare_inputs(X, Y, grid)
    res = _run(in_maps, trace=_trace)
    out = np.empty((B, 3, 128, 128), np.float32)
    for c in range(NCORES):
        out[:, :, :, c * XCOLS:(c + 1) * XCOLS] = res.results[c]["OUT"]
    if _results_out is not None:
        _results_out.append(res)
    return out
